# revision 11
# baseline (speedup 1.0000x reference)
"""Trainium2 Bass kernel for nn_ActionPredictionModel (scatter_memory).

Hybrid sharding over 8 NeuronCores:
  - spec-MLP layer 1 (1801 -> 900) is model-parallel: each core owns a
    113-wide hidden chunk (of 904 padded) and computes it for ALL 64
    graphs; layer 2 partials are summed with a ReduceScatter that lands
    each core its own 8 graphs (b-major rows).
  - everything downstream (value head, pair features, gather, softmax)
    is data-parallel over graphs: 8 graphs (72 nodes) per core.
  - block-diagonal structure: only the 9x9 same-graph pair blocks are
    materialized ([128ch, 648pairs]); indexmask gather via gpsimd
    ap_gather; softmax on DVE/ACT.
Host does only sharding/layout marshalling (transpose, pad, tile-pack,
index remap to the on-device fp layout) and output concatenation.
"""

import numpy as np

# problem dims (hardcoded per contract)
B, NPG, H = 64, 9, 128
SL, SC, BOND, ASL = 1801, 100, 3, 243
NCORES = 8
BPC = B // NCORES            # graphs per core = 8
NODES = BPC * NPG            # nodes per core = 72
PAIRS = BPC * NPG * NPG      # same-graph pairs per core = 648

KT = 15                      # k-tiles over spec dim (14*128 + 9)
HID = 900
HIDP = 904                   # padded hidden (8 * 113)
MCH = HIDP // 8              # per-core hidden chunk = 113
SLP = KT * 128               # padded spec len = 1920

# consts column offsets
OFF_WA2A, OFF_WA2B, OFF_WV1, OFF_WA2C = 0, 128, 256, 384
OFF_WV2, OFF_B2, OFF_BV1, OFF_BV2, OFF_BA2, OFF_BF = 512, 513, 514, 515, 516, 517
OFF_WF, OFF_B1C, OFF_W2C, OFF_EYE = 518, 521, 522, 622
CF = 630

_CACHE = {}
DEBUG_TAPS = False


def _f32(x):
    return np.ascontiguousarray(np.asarray(x), dtype=np.float32)


def _build_nc():
    import concourse.mybir as mybir
    import concourse.tile as tile
    import concourse.bacc as bacc
    import concourse.bass as bass

    f32 = mybir.dt.float32
    i16 = mybir.dt.int16
    Alu = mybir.AluOpType
    Act = mybir.ActivationFunctionType

    nc = bacc.Bacc("TRN2", target_bir_lowering=False, debug=False, num_devices=NCORES)

    consts_d = nc.declare_dram_parameter("consts", [128, CF], f32, isOutput=False)
    acts_d = nc.declare_dram_parameter("acts", [128, KT * B + NODES], f32, isOutput=False)
    w1_d = nc.declare_dram_parameter("w1s", [SL, MCH], f32, isOutput=False)
    mask_d = nc.declare_dram_parameter("mask8", [BPC, ASL], f32, isOutput=False)
    idx_d = nc.declare_dram_parameter("idx16", [128, 16], i16, isOutput=False)
    outp_d = nc.declare_dram_parameter("out_p", [BPC, ASL], f32, isOutput=True)
    outv_d = nc.declare_dram_parameter("out_v", [1, BPC], f32, isOutput=True)
    fp_d = nc.dram_tensor("fp_scratch", [BOND, PAIRS], f32)
    cc_in = nc.dram_tensor("cc_in", [B, SC], f32)
    cc_out = nc.dram_tensor("cc_out", [BPC, SC], f32)

    with tile.TileContext(nc) as tc:
        with (
            tc.tile_pool(name="cpool", bufs=1) as cpool,
            tc.tile_pool(name="w1pool", bufs=4) as w1pool,
            tc.tile_pool(name="ppool", bufs=1, space="PSUM") as ppool,
            tc.tile_pool(name="pab", bufs=1, space="PSUM") as pab,
            tc.tile_pool(name="pshort", bufs=3, space="PSUM") as pshort,
        ):
            # ---- input loads (acts first: it gates the L1 stream) ----
            acts = cpool.tile([128, KT * B + NODES], f32)
            nc.scalar.dma_start(acts[:], acts_d[:])
            consts = cpool.tile([128, CF], f32)
            nc.scalar.dma_start(consts[:], consts_d[:])
            idxs = cpool.tile([128, 16], i16)
            nc.scalar.dma_start(idxs[:], idx_d[:])

            # mask / gather-source tiles (memset first: only rows 16*b are real)
            Xt = cpool.tile([128, ASL], f32, tag="Xt")
            Mt = cpool.tile([128, ASL], f32, tag="Mt")
            nc.vector.memset(Xt[:], 0.0)
            nc.vector.memset(Mt[:], 0.0)
            m_out = bass.AP(Mt[:].tensor, Mt[:].offset, [[16 * ASL, BPC], [1, ASL]])
            nc.scalar.dma_start(m_out, mask_d[:])

            # ACT warm-up: load the Exp table early so the real Exp is cheap
            warm = cpool.tile([1, 1], f32)
            nc.vector.memset(warm[:], 0.0)
            warmo = cpool.tile([1, 1], f32)
            nc.scalar.activation(warmo[:], warm[:], Act.Exp)

            sp = acts[:, 0 : KT * B]               # spT all graphs [128, 15*64]
            nf = acts[:, KT * B : KT * B + NODES]  # nfT own graphs [128, 72]

            # ---- spec MLP layer 1 (model-parallel chunk, all graphs) ----
            # single psum region -> accumulate across k directly in PSUM
            h1p = ppool.tile([MCH, B], f32, tag="h1p")
            for k in range(KT):
                kk = 128 if k < KT - 1 else SL - 128 * (KT - 1)  # 9 for last
                w1k = w1pool.tile([128, MCH], f32, tag="w1k")
                nc.sync.dma_start(w1k[:kk, :], w1_d[128 * k : 128 * k + kk, :])
                nc.tensor.matmul(
                    h1p[:], w1k[:kk, :], sp[:kk, B * k : B * (k + 1)],
                    start=(k == 0), stop=(k == KT - 1),
                )
            h1s = cpool.tile([MCH, B], f32)
            nc.vector.tensor_scalar(h1s[:], h1p[:], consts[:MCH, OFF_B1C : OFF_B1C + 1],
                                    0.0, op0=Alu.add, op1=Alu.max)

            # ---- layer 2 partial (b-major) + ReduceScatter ----
            stbp = pshort.tile([B, SC], f32, tag="sh")
            nc.tensor.matmul(stbp[:], h1s[:], consts[:MCH, OFF_W2C : OFF_W2C + SC],
                             start=True, stop=True)
            stbs = cpool.tile([B, SC], f32)
            nc.vector.tensor_copy(stbs[:], stbp[:])
            nc.sync.dma_start(cc_in[:], stbs[:])
            nc.gpsimd.collective_compute(
                "ReduceScatter", Alu.add, replica_groups=[list(range(NCORES))],
                ins=[cc_in[:]], outs=[cc_out[:]],
            )
            stb8 = cpool.tile([BPC, SC], f32)
            nc.sync.dma_start(stb8[:], cc_out[:])
            # transpose to [100, 8] on PE, then bias+relu
            stps = pshort.tile([SC, BPC], f32, tag="sh")
            nc.tensor.transpose(stps[:], stb8[:], consts[:BPC, OFF_EYE : OFF_EYE + BPC])
            sTs = cpool.tile([SC, BPC], f32)
            nc.vector.tensor_scalar(sTs[:], stps[:], consts[:SC, OFF_B2 : OFF_B2 + 1],
                                    0.0, op0=Alu.add, op1=Alu.max)

            # ---- value head ----
            ro = cpool.tile([128, BPC], f32)  # readoutT = per-graph sum of 9 node cols
            nc.vector.reduce_sum(ro[:], nf.rearrange("p (b n) -> p b n", n=NPG), axis=mybir.AxisListType.X)
            y1 = pshort.tile([64, BPC], f32, tag="sh")
            nc.tensor.matmul(y1[:], consts[:, OFF_WV1 : OFF_WV1 + 64], ro[:], start=True, stop=False)
            nc.tensor.matmul(y1[:], consts[:SC, OFF_WV1 + 64 : OFF_WV1 + 128], sTs[:], start=False, stop=True)
            y1s = cpool.tile([64, BPC], f32)
            nc.vector.tensor_scalar(y1s[:], y1[:], consts[:64, OFF_BV1 : OFF_BV1 + 1],
                                    0.0, op0=Alu.add, op1=Alu.max)
            vps = pshort.tile([1, BPC], f32, tag="sh")
            nc.tensor.matmul(vps[:], consts[:64, OFF_WV2 : OFF_WV2 + 1], y1s[:], start=True, stop=True)
            vs = cpool.tile([1, BPC], f32)
            nc.vector.tensor_scalar_add(vs[:], vps[:], consts[:1, OFF_BV2 : OFF_BV2 + 1])
            nc.sync.dma_start(outv_d[:], vs[:])

            # ---- pair features: hT[c, (b,i,j)] ----
            nfr = cpool.tile([128, NODES], f32)
            nc.vector.tensor_scalar_max(nfr[:], nf, 0.0)
            aips = pab.tile([128, NODES], f32, tag="aips")
            nc.tensor.matmul(aips[:], consts[:, OFF_WA2A : OFF_WA2A + 128], nfr[:], start=True, stop=True)
            bjps = pab.tile([128, NODES], f32, tag="bjps")
            nc.tensor.matmul(bjps[:], consts[:, OFF_WA2B : OFF_WA2B + 128], nfr[:], start=True, stop=True)
            bjs = cpool.tile([128, NODES], f32)
            nc.vector.tensor_copy(bjs[:], bjps[:])
            dps = pshort.tile([128, BPC], f32, tag="sh")
            nc.tensor.matmul(dps[:], consts[:SC, OFF_WA2C : OFF_WA2C + 128], sTs[:], start=True, stop=True)
            dt2 = cpool.tile([128, BPC], f32)
            nc.vector.tensor_scalar_add(dt2[:], dps[:], consts[:, OFF_BA2 : OFF_BA2 + 1])
            ai2 = cpool.tile([128, NODES], f32)
            nc.vector.tensor_tensor(
                ai2[:].rearrange("p (b i) -> p b i", i=NPG),
                aips[:].rearrange("p (b i) -> p b i", i=NPG),
                dt2[:].unsqueeze(2).broadcast_to([128, BPC, NPG]),
                op=Alu.add,
            )
            hT = cpool.tile([128, PAIRS], f32)
            nc.vector.tensor_tensor(
                hT[:].rearrange("p (b i j) -> p b i j", i=NPG, j=NPG),
                ai2[:].rearrange("p (b i) -> p b i", i=NPG).unsqueeze(3).broadcast_to([128, BPC, NPG, NPG]),
                bjs[:].rearrange("p (b j) -> p b j", j=NPG).unsqueeze(2).broadcast_to([128, BPC, NPG, NPG]),
                op=Alu.add,
            )
            nc.vector.tensor_scalar_max(hT[:], hT[:], 0.0)

            # ---- saf: fp[t, pair] = Wf.T @ relu(hT) + bf ----
            fp1 = pshort.tile([BOND, PAIRS // 2], f32, tag="sh")
            fp2 = pshort.tile([BOND, PAIRS // 2], f32, tag="sh")
            nc.tensor.matmul(fp1[:], consts[:, OFF_WF : OFF_WF + BOND], hT[:, : PAIRS // 2], start=True, stop=True)
            nc.tensor.matmul(fp2[:], consts[:, OFF_WF : OFF_WF + BOND], hT[:, PAIRS // 2 :], start=True, stop=True)
            fps = cpool.tile([BOND, PAIRS], f32)
            nc.vector.tensor_scalar_add(fps[:, : PAIRS // 2], fp1[:], consts[:BOND, OFF_BF : OFF_BF + 1])
            nc.vector.tensor_scalar_add(fps[:, PAIRS // 2 :], fp2[:], consts[:BOND, OFF_BF : OFF_BF + 1])

            # bounce through DRAM to regroup [3, 648] -> X[16b, t*81+e]
            nc.sync.dma_start(fp_d[:], fps[:])
            x_out = bass.AP(Xt[:].tensor, Xt[:].offset, [[16 * ASL, BPC], [1, ASL]])
            x_in = bass.AP(fp_d[:].tensor, 0, [[81, BPC], [PAIRS, BOND], [1, 81]])
            nc.sync.dma_start(x_out, x_in)

            # ---- gather + masked softmax ----
            G = cpool.tile([128, 256], f32)
            nc.gpsimd.ap_gather(G[:], Xt[:], idxs[:], channels=128, num_elems=ASL, d=1, num_idxs=256)
            X2 = cpool.tile([128, ASL], f32)
            nc.vector.tensor_tensor(X2[:], G[:, :ASL], Mt[:], op=Alu.add)
            nmx = cpool.tile([128, 1], f32)
            nc.vector.reduce_max(nmx[:], X2[:], axis=mybir.AxisListType.X, negate=True)
            E = cpool.tile([128, ASL], f32)
            sums = cpool.tile([128, 1], f32)
            nc.scalar.activation(E[:], X2[:], Act.Exp, bias=nmx[:], accum_out=sums[:])
            rc = cpool.tile([128, 1], f32)
            nc.vector.reciprocal(rc[:], sums[:])
            OU = cpool.tile([128, ASL], f32)
            nc.vector.tensor_scalar_mul(OU[:], E[:], rc[:])
            o_in = bass.AP(OU[:].tensor, OU[:].offset, [[16 * ASL, BPC], [1, ASL]])
            nc.sync.dma_start(outp_d[:], o_in)

            if DEBUG_TAPS:
                taps = {
                    "t_h1s": h1s, "t_stbs": stbs, "t_stb8": stb8, "t_sTs": sTs,
                    "t_ro": ro, "t_y1s": y1s, "t_nfr": nfr, "t_dt2": dt2,
                    "t_ai2": ai2, "t_bjs": bjs, "t_hT": hT, "t_fps": fps,
                    "t_Xt": Xt, "t_G": G, "t_X2": X2, "t_sums": sums,
                }
                for tname, ttile in taps.items():
                    shp = list(ttile[:].shape)
                    td = nc.declare_dram_parameter(tname, shp, f32, isOutput=True)
                    nc.sync.dma_start(td[:], ttile[:])

    nc.compile()
    return nc


def _marshal(node_features, specs, mask, indexmask, W1, b1, W2, b2,
             Wv1, bv1, Wv2, bv2, Wa2, ba2, Wf, bf):
    """Host-side sharding + layout packing. Returns in_maps (one per core)."""
    w1p = np.zeros((SL, HIDP), np.float32)
    w1p[:, :HID] = W1
    b1p = np.zeros(HIDP, np.float32)
    b1p[:HID] = b1
    w2p = np.zeros((HIDP, SC), np.float32)
    w2p[:HID] = W2

    cbase = np.zeros((128, CF), np.float32)
    cbase[:, OFF_WA2A : OFF_WA2A + 128] = Wa2[0:128]
    cbase[:, OFF_WA2B : OFF_WA2B + 128] = Wa2[128:256]
    cbase[:, OFF_WV1 : OFF_WV1 + 64] = Wv1[0:128]
    cbase[:100, OFF_WV1 + 64 : OFF_WV1 + 128] = Wv1[128:228]
    cbase[:100, OFF_WA2C : OFF_WA2C + 128] = Wa2[256:356]
    cbase[:64, OFF_WV2] = Wv2[:, 0]
    cbase[:100, OFF_B2] = b2
    cbase[:64, OFF_BV1] = bv1
    cbase[:1, OFF_BV2] = bv2
    cbase[:, OFF_BA2] = ba2
    cbase[:BOND, OFF_BF] = bf
    cbase[:, OFF_WF : OFF_WF + BOND] = Wf
    cbase[:BPC, OFF_EYE : OFF_EYE + BPC] = np.eye(BPC, dtype=np.float32)

    # spec transposed + k-tiled, all graphs: spT[p, 64k+b] = spec[b, 128k+p]
    spc = np.zeros((B, SLP), np.float32)
    spc[:, :SL] = specs[:, 0, :]
    spT = spc.reshape(B, KT, 128).transpose(2, 1, 0).reshape(128, KT * B)

    # index remap to device fp layout (t-major): v -> (v%3)*81 + v//3,
    # then wrap per-graph lists across the 16 partitions of its group
    v = indexmask.astype(np.int64)
    newidx = ((v % BOND) * (NPG * NPG) + v // BOND).astype(np.int16)  # [64, 243]

    in_maps = []
    for c in range(NCORES):
        gsl = slice(c * BPC, (c + 1) * BPC)
        nsl = slice(c * NODES, (c + 1) * NODES)
        consts = cbase.copy()
        consts[:MCH, OFF_B1C] = b1p[MCH * c : MCH * (c + 1)]
        consts[:MCH, OFF_W2C : OFF_W2C + SC] = w2p[MCH * c : MCH * (c + 1)]
        acts = np.zeros((128, KT * B + NODES), np.float32)
        acts[:, 0 : KT * B] = spT
        acts[:, KT * B :] = node_features[nsl].T
        padidx = np.zeros((BPC, 256), np.int16)
        padidx[:, :ASL] = newidx[gsl]
        idx16 = padidx.reshape(BPC, 16, 16).transpose(0, 2, 1).reshape(128, 16)
        in_maps.append({
            "consts": consts,
            "acts": acts,
            "w1s": np.ascontiguousarray(w1p[:, MCH * c : MCH * (c + 1)]),
            "mask8": np.ascontiguousarray(mask[gsl], np.float32),
            "idx16": np.ascontiguousarray(idx16),
        })
    return in_maps


def _run(inputs, trace=False):
    from concourse.bass_utils import run_bass_kernel_spmd

    if "nc" not in _CACHE:
        _CACHE["nc"] = _build_nc()
    nc = _CACHE["nc"]

    in_maps = _marshal(
        _f32(inputs["node_features"]), _f32(inputs["specs"]),
        _f32(inputs["mask"]), np.asarray(inputs["indexmask"]),
        _f32(inputs["W1"]), _f32(inputs["b1"]), _f32(inputs["W2"]), _f32(inputs["b2"]),
        _f32(inputs["Wv1"]), _f32(inputs["bv1"]), _f32(inputs["Wv2"]), _f32(inputs["bv2"]),
        _f32(inputs["Wa2"]), _f32(inputs["ba2"]), _f32(inputs["Wf"]), _f32(inputs["bf"]),
    )
    res = run_bass_kernel_spmd(nc, in_maps, core_ids=list(range(NCORES)), trace=trace)
    probs = np.concatenate([res.results[c]["out_p"] for c in range(NCORES)], axis=0)
    v = np.concatenate([res.results[c]["out_v"][0] for c in range(NCORES)])[:, None]
    return (probs, v.astype(np.float32)), res


def kernel(**inputs):
    (probs, v), _ = _run(inputs, trace=False)
    return probs, v


# revision 13
# speedup vs baseline: 1.6694x; 1.6694x over previous
"""Trainium2 Bass kernel for nn_ActionPredictionModel (scatter_memory).

Data-parallel over graphs: 8 graphs (72 nodes) per NeuronCore, weights
replicated (collectives measured ~80us launch-skew here, so none used).
Per core:
  - spec MLP layer 1 with W1 as the *moving* operand (b-major output):
    fp32 PE cost is ~2.2ns/col of moving stream, vs ~3.7ns/col as
    stationary, so stationary is the tiny spec tile (8 cols) and all of
    W1 streams through as rhs. b1 is folded in via a constant-1 spec row.
    h1 [8, 904] is then relu'd and PE-transposed (8x [8,113]->[113,8])
    for the hid-major layer 2.
  - value head (sum-pool readout + spec -> scalar)
  - pair action features; block-diagonal structure: only the 9x9
    same-graph pair blocks are materialized ([128ch, 648pairs])
  - per-graph flatten + indexmask gather (gpsimd ap_gather, ucode
    warmed early) + softmax (DVE + one ACT Exp, table warmed early)
Host does only sharding/layout marshalling (transpose, pad, tile-pack,
index remap to the on-device fp layout) and output concatenation.
"""

import numpy as np

# problem dims (hardcoded per contract)
B, NPG, H = 64, 9, 128
SL, SC, BOND, ASL = 1801, 100, 3, 243
NCORES = 8
BPC = B // NCORES            # graphs per core = 8
NODES = BPC * NPG            # nodes per core = 72
PAIRS = BPC * NPG * NPG      # same-graph pairs per core = 648

KT = 15                      # k-tiles over spec dim (14*128 + 10(incl bias row))
HID = 900
HIDP = 904                   # padded hidden (8 * 113)
MCH = HIDP // 8              # hid chunk for transposes / L2 = 113
HHALF = HIDP // 2            # 452 (psum bank-sized moving chunks)

# consts column offsets
OFF_WA2A, OFF_WA2B, OFF_WV1, OFF_WA2C = 0, 128, 256, 384
OFF_WV2, OFF_B2, OFF_BV1, OFF_BV2, OFF_BA2, OFF_BF = 512, 513, 514, 515, 516, 517
OFF_WF, OFF_EYE, OFF_W2T = 518, 521, 529
CF = 529 + 800

_CACHE = {}
DEBUG_TAPS = False


def _f32(x):
    return np.ascontiguousarray(np.asarray(x), dtype=np.float32)


def _build_nc():
    import concourse.mybir as mybir
    import concourse.tile as tile
    import concourse.bacc as bacc
    import concourse.bass as bass

    f32 = mybir.dt.float32
    i16 = mybir.dt.int16
    Alu = mybir.AluOpType
    Act = mybir.ActivationFunctionType

    nc = bacc.Bacc("TRN2", target_bir_lowering=False, debug=False, num_devices=1)

    consts_d = nc.declare_dram_parameter("consts", [128, CF], f32, isOutput=False)
    acts_d = nc.declare_dram_parameter("acts", [128, KT * BPC + NODES], f32, isOutput=False)
    w1a_d = nc.declare_dram_parameter("w1a", [128, 7 * HIDP], f32, isOutput=False)
    w1b_d = nc.declare_dram_parameter("w1b", [128, 7 * HIDP], f32, isOutput=False)
    w1t_d = nc.declare_dram_parameter("w1t", [10, HIDP], f32, isOutput=False)
    mask_d = nc.declare_dram_parameter("mask8", [BPC, ASL], f32, isOutput=False)
    idx_d = nc.declare_dram_parameter("idx16", [128, 16], i16, isOutput=False)
    outp_d = nc.declare_dram_parameter("out_p", [BPC, ASL], f32, isOutput=True)
    outv_d = nc.declare_dram_parameter("out_v", [1, BPC], f32, isOutput=True)
    fp_d = nc.dram_tensor("fp_scratch", [BOND, PAIRS], f32)

    with tile.TileContext(nc) as tc:
        with (
            tc.tile_pool(name="cpool", bufs=1) as cpool,
            tc.tile_pool(name="pab", bufs=1, space="PSUM") as pab,
            tc.tile_pool(name="psh", bufs=3, space="PSUM") as psh,
        ):
            # ---- input loads: acts + W1 gate the stream; consts/idx/mask on gpsimd ----
            acts = cpool.tile([128, KT * BPC + NODES], f32)
            nc.scalar.dma_start(acts[:], acts_d[:])
            w1a = cpool.tile([128, 7 * HIDP], f32)
            nc.sync.dma_start(w1a[:], w1a_d[:])
            w1b = cpool.tile([128, 7 * HIDP], f32)
            nc.scalar.dma_start(w1b[:], w1b_d[:])
            w1t = cpool.tile([10, HIDP], f32)
            nc.sync.dma_start(w1t[:], w1t_d[:])
            consts = cpool.tile([128, CF], f32)
            nc.gpsimd.dma_start(consts[:], consts_d[:])
            idxs = cpool.tile([128, 16], i16)
            nc.gpsimd.dma_start(idxs[:], idx_d[:])

            # mask / gather-source tiles (memset first: only rows 16*b are real)
            Xt = cpool.tile([128, ASL], f32, tag="Xt")
            Mt = cpool.tile([128, ASL], f32, tag="Mt")
            nc.vector.memset(Xt[:], 0.0)
            nc.vector.memset(Mt[:], 0.0)
            m_out = bass.AP(Mt[:].tensor, Mt[:].offset, [[16 * ASL, BPC], [1, ASL]])
            nc.gpsimd.dma_start(m_out, mask_d[:])

            # warm-ups: ACT Exp table + gpsimd ap_gather ucode (hide under stream)
            warm = cpool.tile([1, 1], f32)
            nc.vector.memset(warm[:], 0.0)
            warmo = cpool.tile([1, 1], f32)
            nc.scalar.activation(warmo[:], warm[:], Act.Exp)
            gwi = cpool.tile([16, 4], f32)
            gwx = cpool.tile([16, 1], i16)
            gwo = cpool.tile([16, 16], f32)
            nc.vector.memset(gwi[:], 0.0)
            nc.vector.memset(gwx[:], 0)
            nc.gpsimd.ap_gather(gwo[:], gwi[:], gwx[:], channels=16, num_elems=4, d=1, num_idxs=16)

            sp = acts[:, 0 : KT * BPC]               # spT own graphs [128, 15*8]
            nf = acts[:, KT * BPC : KT * BPC + NODES]  # nfT own graphs [128, 72]

            # ---- spec MLP layer 1: b-major, W1 moving, accumulate in PSUM ----
            h1bs = cpool.tile([BPC, HIDP], f32)
            with tc.tile_pool(name="ph1", bufs=2, space="PSUM") as ph1:
                h1p0 = ph1.tile([BPC, HHALF], f32, tag="h1b")
                h1p1 = ph1.tile([BPC, HHALF], f32, tag="h1b")
                for k in range(KT):
                    kk = 128 if k < KT - 1 else 10  # last tile: 9 spec rows + bias row
                    if k < 7:
                        wsl = w1a[:kk, HIDP * k : HIDP * (k + 1)]
                    elif k < 14:
                        wsl = w1b[:kk, HIDP * (k - 7) : HIDP * (k - 6)]
                    else:
                        wsl = w1t[:kk, :]
                    lhs = sp[:kk, BPC * k : BPC * (k + 1)]
                    nc.tensor.matmul(h1p0[:], lhs, wsl[:, :HHALF],
                                     start=(k == 0), stop=(k == KT - 1))
                    nc.tensor.matmul(h1p1[:], lhs, wsl[:, HHALF:],
                                     start=(k == 0), stop=(k == KT - 1))
                # relu (bias already folded via the constant-1 spec row)
                nc.vector.tensor_scalar_max(h1bs[:, :HHALF], h1p0[:], 0.0)
                nc.vector.tensor_scalar_max(h1bs[:, HHALF:], h1p1[:], 0.0)

            with tc.tile_pool(name="ptr", bufs=2, space="PSUM") as ptr:
                # ---- transpose h1 to hid-major: 8x PE transpose [8,113]->[113,8] ----
                eye = consts[:BPC, OFF_EYE : OFF_EYE + BPC]
                h1ts = cpool.tile([MCH, 64], f32)
                for j in range(8):
                    tp = ptr.tile([MCH, BPC], f32, tag="tr")
                    nc.tensor.transpose(tp[:], h1bs[:, MCH * j : MCH * (j + 1)], eye)
                    nc.vector.tensor_copy(h1ts[:, BPC * j : BPC * (j + 1)], tp[:])

                # ---- layer 2: sT[q, b] accumulated over 8 hid chunks ----
                sps = psh.tile([SC, BPC], f32, tag="sh")
                for j in range(8):
                    nc.tensor.matmul(
                        sps[:],
                        consts[:MCH, OFF_W2T + SC * j : OFF_W2T + SC * (j + 1)],
                        h1ts[:, BPC * j : BPC * (j + 1)],
                        start=(j == 0), stop=(j == 7),
                    )
                sTs = cpool.tile([SC, BPC], f32)
                nc.vector.tensor_scalar(sTs[:], sps[:], consts[:SC, OFF_B2 : OFF_B2 + 1],
                                        0.0, op0=Alu.add, op1=Alu.max)

                # ---- value head ----
                ro = cpool.tile([128, BPC], f32)
                nc.vector.reduce_sum(ro[:], nf.rearrange("p (b n) -> p b n", n=NPG),
                                     axis=mybir.AxisListType.X)
                y1 = psh.tile([64, BPC], f32, tag="sh")
                nc.tensor.matmul(y1[:], consts[:, OFF_WV1 : OFF_WV1 + 64], ro[:], start=True, stop=False)
                nc.tensor.matmul(y1[:], consts[:SC, OFF_WV1 + 64 : OFF_WV1 + 128], sTs[:], start=False, stop=True)
                y1s = cpool.tile([64, BPC], f32)
                nc.vector.tensor_scalar(y1s[:], y1[:], consts[:64, OFF_BV1 : OFF_BV1 + 1],
                                        0.0, op0=Alu.add, op1=Alu.max)
                vps = psh.tile([1, BPC], f32, tag="sh")
                nc.tensor.matmul(vps[:], consts[:64, OFF_WV2 : OFF_WV2 + 1], y1s[:], start=True, stop=True)
                vs = cpool.tile([1, BPC], f32)
                nc.vector.tensor_scalar_add(vs[:], vps[:], consts[:1, OFF_BV2 : OFF_BV2 + 1])
                nc.sync.dma_start(outv_d[:], vs[:])

                # ---- pair features: hT[c, (b,i,j)] ----
                nfr = cpool.tile([128, NODES], f32)
                nc.vector.tensor_scalar_max(nfr[:], nf, 0.0)
                aips = pab.tile([128, NODES], f32, tag="aips")
                nc.tensor.matmul(aips[:], consts[:, OFF_WA2A : OFF_WA2A + 128], nfr[:], start=True, stop=True)
                bjps = pab.tile([128, NODES], f32, tag="bjps")
                nc.tensor.matmul(bjps[:], consts[:, OFF_WA2B : OFF_WA2B + 128], nfr[:], start=True, stop=True)
                bjs = cpool.tile([128, NODES], f32)
                nc.vector.tensor_copy(bjs[:], bjps[:])
                dps = psh.tile([128, BPC], f32, tag="sh")
                nc.tensor.matmul(dps[:], consts[:SC, OFF_WA2C : OFF_WA2C + 128], sTs[:], start=True, stop=True)
                dt2 = cpool.tile([128, BPC], f32)
                nc.vector.tensor_scalar_add(dt2[:], dps[:], consts[:, OFF_BA2 : OFF_BA2 + 1])
                ai2 = cpool.tile([128, NODES], f32)
                nc.vector.tensor_tensor(
                    ai2[:].rearrange("p (b i) -> p b i", i=NPG),
                    aips[:].rearrange("p (b i) -> p b i", i=NPG),
                    dt2[:].unsqueeze(2).broadcast_to([128, BPC, NPG]),
                    op=Alu.add,
                )
                hT = cpool.tile([128, PAIRS], f32)
                nc.vector.tensor_tensor(
                    hT[:].rearrange("p (b i j) -> p b i j", i=NPG, j=NPG),
                    ai2[:].rearrange("p (b i) -> p b i", i=NPG).unsqueeze(3).broadcast_to([128, BPC, NPG, NPG]),
                    bjs[:].rearrange("p (b j) -> p b j", j=NPG).unsqueeze(2).broadcast_to([128, BPC, NPG, NPG]),
                    op=Alu.add,
                )
                nc.vector.tensor_scalar_max(hT[:], hT[:], 0.0)

                # ---- saf: fp[t, pair] = Wf.T @ relu(hT) + bf ----
                fp1 = ptr.tile([BOND, PAIRS // 2], f32, tag="tr")
                fp2 = ptr.tile([BOND, PAIRS // 2], f32, tag="tr")
                nc.tensor.matmul(fp1[:], consts[:, OFF_WF : OFF_WF + BOND], hT[:, : PAIRS // 2], start=True, stop=True)
                nc.tensor.matmul(fp2[:], consts[:, OFF_WF : OFF_WF + BOND], hT[:, PAIRS // 2 :], start=True, stop=True)
                fps = cpool.tile([BOND, PAIRS], f32)
                nc.vector.tensor_scalar_add(fps[:, : PAIRS // 2], fp1[:], consts[:BOND, OFF_BF : OFF_BF + 1])
                nc.vector.tensor_scalar_add(fps[:, PAIRS // 2 :], fp2[:], consts[:BOND, OFF_BF : OFF_BF + 1])

                # regroup [3, 648] -> X[16b, t*81+e] via a DRAM bounce
                nc.sync.dma_start(fp_d[:], fps[:])
                x_out = bass.AP(Xt[:].tensor, Xt[:].offset, [[16 * ASL, BPC], [1, ASL]])
                x_in = bass.AP(fp_d[:].tensor, 0, [[81, BPC], [PAIRS, BOND], [1, 81]])
                nc.sync.dma_start(x_out, x_in)

                # ---- gather + masked softmax ----
                G = cpool.tile([128, 256], f32)
                nc.gpsimd.ap_gather(G[:], Xt[:], idxs[:], channels=128, num_elems=ASL, d=1, num_idxs=256)
                X2 = cpool.tile([128, ASL], f32)
                nc.vector.tensor_tensor(X2[:], G[:, :ASL], Mt[:], op=Alu.add)
                nmx = cpool.tile([128, 1], f32)
                nc.vector.reduce_max(nmx[:], X2[:], axis=mybir.AxisListType.X, negate=True)
                E = cpool.tile([128, ASL], f32)
                sums = cpool.tile([128, 1], f32)
                nc.scalar.activation(E[:], X2[:], Act.Exp, bias=nmx[:], accum_out=sums[:])
                rc = cpool.tile([128, 1], f32)
                nc.vector.reciprocal(rc[:], sums[:])
                OU = cpool.tile([128, ASL], f32)
                nc.vector.tensor_scalar_mul(OU[:], E[:], rc[:])
                o_in = bass.AP(OU[:].tensor, OU[:].offset, [[16 * ASL, BPC], [1, ASL]])
                nc.sync.dma_start(outp_d[:], o_in)

                if DEBUG_TAPS:
                    taps = {
                        "t_h1bs": h1bs, "t_h1ts": h1ts, "t_sTs": sTs, "t_ro": ro,
                        "t_y1s": y1s, "t_nfr": nfr, "t_dt2": dt2, "t_ai2": ai2,
                        "t_bjs": bjs, "t_hT": hT, "t_fps": fps, "t_Xt": Xt,
                        "t_G": G, "t_X2": X2, "t_sums": sums,
                    }
                    for tname, ttile in taps.items():
                        shp = list(ttile[:].shape)
                        td = nc.declare_dram_parameter(tname, shp, f32, isOutput=True)
                        nc.sync.dma_start(td[:], ttile[:])

    nc.compile()
    return nc


def _marshal(node_features, specs, mask, indexmask, W1, b1, W2, b2,
             Wv1, bv1, Wv2, bv2, Wa2, ba2, Wf, bf):
    """Host-side sharding + layout packing. Returns in_maps (one per core)."""
    # W1 with b1 folded as an extra (constant-1-input) row, k-tile packed
    w1p = np.zeros((1802, HIDP), np.float32)
    w1p[:SL, :HID] = W1
    w1p[SL, :HID] = b1
    w1a = np.ascontiguousarray(w1p[:896].reshape(7, 128, HIDP).transpose(1, 0, 2).reshape(128, 7 * HIDP))
    w1b = np.ascontiguousarray(w1p[896:1792].reshape(7, 128, HIDP).transpose(1, 0, 2).reshape(128, 7 * HIDP))
    w1t = np.ascontiguousarray(w1p[1792:1802])
    w2p = np.zeros((HIDP, SC), np.float32)
    w2p[:HID] = W2

    consts = np.zeros((128, CF), np.float32)
    consts[:, OFF_WA2A : OFF_WA2A + 128] = Wa2[0:128]
    consts[:, OFF_WA2B : OFF_WA2B + 128] = Wa2[128:256]
    consts[:, OFF_WV1 : OFF_WV1 + 64] = Wv1[0:128]
    consts[:100, OFF_WV1 + 64 : OFF_WV1 + 128] = Wv1[128:228]
    consts[:100, OFF_WA2C : OFF_WA2C + 128] = Wa2[256:356]
    consts[:64, OFF_WV2] = Wv2[:, 0]
    consts[:100, OFF_B2] = b2
    consts[:64, OFF_BV1] = bv1
    consts[:1, OFF_BV2] = bv2
    consts[:, OFF_BA2] = ba2
    consts[:BOND, OFF_BF] = bf
    consts[:, OFF_WF : OFF_WF + BOND] = Wf
    consts[:BPC, OFF_EYE : OFF_EYE + BPC] = np.eye(BPC, dtype=np.float32)
    consts[:MCH, OFF_W2T : OFF_W2T + 800] = w2p.reshape(8, MCH, SC).transpose(1, 0, 2).reshape(MCH, 800)

    # index remap to device fp layout (t-major): v -> (v%3)*81 + v//3,
    # then wrap per-graph lists across the 16 partitions of its group
    v = indexmask.astype(np.int64)
    newidx = ((v % BOND) * (NPG * NPG) + v // BOND).astype(np.int16)  # [64, 243]

    in_maps = []
    for c in range(NCORES):
        gsl = slice(c * BPC, (c + 1) * BPC)
        nsl = slice(c * NODES, (c + 1) * NODES)
        # spec transposed + k-tiled + constant-1 bias row (row 1801)
        spc = np.zeros((BPC, KT * 128), np.float32)
        spc[:, :SL] = specs[gsl, 0, :]
        spc[:, SL] = 1.0
        spT = spc.reshape(BPC, KT, 128).transpose(2, 1, 0).reshape(128, KT * BPC)
        acts = np.zeros((128, KT * BPC + NODES), np.float32)
        acts[:, 0 : KT * BPC] = spT
        acts[:, KT * BPC :] = node_features[nsl].T
        padidx = np.zeros((BPC, 256), np.int16)
        padidx[:, :ASL] = newidx[gsl]
        idx16 = padidx.reshape(BPC, 16, 16).transpose(0, 2, 1).reshape(128, 16)
        in_maps.append({
            "consts": consts,
            "acts": acts,
            "w1a": w1a, "w1b": w1b, "w1t": w1t,
            "mask8": np.ascontiguousarray(mask[gsl], np.float32),
            "idx16": np.ascontiguousarray(idx16),
        })
    return in_maps


def _run(inputs, trace=False):
    from concourse.bass_utils import run_bass_kernel_spmd

    if "nc" not in _CACHE:
        _CACHE["nc"] = _build_nc()
    nc = _CACHE["nc"]

    in_maps = _marshal(
        _f32(inputs["node_features"]), _f32(inputs["specs"]),
        _f32(inputs["mask"]), np.asarray(inputs["indexmask"]),
        _f32(inputs["W1"]), _f32(inputs["b1"]), _f32(inputs["W2"]), _f32(inputs["b2"]),
        _f32(inputs["Wv1"]), _f32(inputs["bv1"]), _f32(inputs["Wv2"]), _f32(inputs["bv2"]),
        _f32(inputs["Wa2"]), _f32(inputs["ba2"]), _f32(inputs["Wf"]), _f32(inputs["bf"]),
    )
    res = run_bass_kernel_spmd(nc, in_maps, core_ids=list(range(NCORES)), trace=trace)
    probs = np.concatenate([res.results[c]["out_p"] for c in range(NCORES)], axis=0)
    v = np.concatenate([res.results[c]["out_v"][0] for c in range(NCORES)])[:, None]
    return (probs, v.astype(np.float32)), res


def kernel(**inputs):
    (probs, v), _ = _run(inputs, trace=False)
    return probs, v


# revision 15
# speedup vs baseline: 1.9746x; 1.1828x over previous
"""Trainium2 Bass kernel for nn_ActionPredictionModel (scatter_memory).

Data-parallel over graphs: 8 graphs (72 nodes) per NeuronCore, weights
replicated (collectives measured ~80us launch-skew here, so none used).
Per core:
  - spec MLP layer 1 with W1 as the *moving* operand (b-major output):
    fp32 PE cost is ~2.2ns/col of moving stream, vs ~3.7ns/col as
    stationary, so stationary is the tiny spec tile (8 cols) and all of
    W1 streams through as rhs. b1 is folded in via a constant-1 spec row.
    h1 [8, 904] is then relu'd and PE-transposed (8x [8,113]->[113,8])
    for the hid-major layer 2.
  - value head (sum-pool readout + spec -> scalar)
  - pair action features; block-diagonal structure: only the 9x9
    same-graph pair blocks are materialized ([128ch, 648pairs])
  - per-graph flatten + indexmask gather (gpsimd ap_gather, ucode
    warmed early) + softmax (DVE + one ACT Exp, table warmed early)
Host does only sharding/layout marshalling (transpose, pad, tile-pack,
index remap to the on-device fp layout) and output concatenation.
"""

import numpy as np

# problem dims (hardcoded per contract)
B, NPG, H = 64, 9, 128
SL, SC, BOND, ASL = 1801, 100, 3, 243
NCORES = 8
BPC = B // NCORES            # graphs per core = 8
NODES = BPC * NPG            # nodes per core = 72
PAIRS = BPC * NPG * NPG      # same-graph pairs per core = 648

KT = 15                      # k-tiles over spec dim (14*128 + 10(incl bias row))
HID = 900
HIDP = 904                   # padded hidden (8 * 113)
MCH = HIDP // 8              # hid chunk for transposes / L2 = 113
HHALF = HIDP // 2            # 452 (psum bank-sized moving chunks)

# consts column offsets
OFF_WA2A, OFF_WA2B, OFF_WV1, OFF_WA2C = 0, 128, 256, 384
OFF_WV2, OFF_B2, OFF_BV1, OFF_BV2, OFF_BA2, OFF_BF = 512, 513, 514, 515, 516, 517
OFF_WF, OFF_EYE, OFF_W2T = 518, 521, 529
CF = 529 + 800

_CACHE = {}
DEBUG_TAPS = False


def _f32(x):
    return np.ascontiguousarray(np.asarray(x), dtype=np.float32)


def _build_nc():
    import concourse.mybir as mybir
    import concourse.tile as tile
    import concourse.bacc as bacc
    import concourse.bass as bass

    f32 = mybir.dt.float32
    i16 = mybir.dt.int16
    Alu = mybir.AluOpType
    Act = mybir.ActivationFunctionType

    nc = bacc.Bacc("TRN2", target_bir_lowering=False, debug=False, num_devices=1)

    consts_d = nc.declare_dram_parameter("consts", [128, CF], f32, isOutput=False)
    acts_d = nc.declare_dram_parameter("acts", [128, KT * BPC + NODES], f32, isOutput=False)
    w1k_d = [nc.declare_dram_parameter(f"w1k{k}", [128 if k < KT - 1 else 10, HIDP], f32, isOutput=False)
             for k in range(KT)]
    mask_d = nc.declare_dram_parameter("mask8", [BPC, ASL], f32, isOutput=False)
    idx_d = nc.declare_dram_parameter("idx16", [128, 16], i16, isOutput=False)
    outp_d = nc.declare_dram_parameter("out_p", [BPC, ASL], f32, isOutput=True)
    outv_d = nc.declare_dram_parameter("out_v", [1, BPC], f32, isOutput=True)
    fp_d = nc.dram_tensor("fp_scratch", [BOND, PAIRS], f32)

    with tile.TileContext(nc) as tc:
        with (
            tc.tile_pool(name="cpool", bufs=1) as cpool,
            tc.tile_pool(name="pab", bufs=1, space="PSUM") as pab,
            tc.tile_pool(name="psh", bufs=2, space="PSUM") as psh,
            tc.tile_pool(name="ph1", bufs=2, space="PSUM") as ph1,
            tc.tile_pool(name="ptr", bufs=2, space="PSUM") as ptr,
        ):
            # ---- input loads: acts + per-k W1 chunks gate the stream ----
            acts = cpool.tile([128, KT * BPC + NODES], f32)
            nc.scalar.dma_start(acts[:], acts_d[:])
            idxs = cpool.tile([128, 16], i16)
            nc.sync.dma_start(idxs[:], idx_d[:])
            w1ts = []
            for k in range(KT):
                kk = 128 if k < KT - 1 else 10
                wt = cpool.tile([kk, HIDP], f32, tag=f"w1k{k}")
                eng = nc.sync if k % 2 == 0 else nc.scalar
                eng.dma_start(wt[:], w1k_d[k][:])
                w1ts.append(wt)
            consts = cpool.tile([128, CF], f32)
            nc.sync.dma_start(consts[:], consts_d[:])

            # mask / gather-source tiles (memset first: only rows 16*b are real)
            Xt = cpool.tile([128, ASL], f32, tag="Xt")
            Mt = cpool.tile([128, ASL], f32, tag="Mt")
            nc.vector.memset(Xt[:], 0.0)
            nc.vector.memset(Mt[:], 0.0)
            m_out = bass.AP(Mt[:].tensor, Mt[:].offset, [[16 * ASL, BPC], [1, ASL]])
            nc.scalar.dma_start(m_out, mask_d[:])

            # warm-ups: ACT Exp table + gpsimd ap_gather ucode (hide under stream)
            warm = cpool.tile([1, 1], f32)
            nc.vector.memset(warm[:], 0.0)
            warmo = cpool.tile([1, 1], f32)
            nc.scalar.activation(warmo[:], warm[:], Act.Exp)
            gwi = cpool.tile([16, 4], f32)
            gwx = cpool.tile([16, 1], i16)
            gwo = cpool.tile([16, 16], f32)
            nc.vector.memset(gwi[:], 0.0)
            nc.vector.memset(gwx[:], 0)
            nc.gpsimd.ap_gather(gwo[:], gwi[:], gwx[:], channels=16, num_elems=4, d=1, num_idxs=16)

            sp = acts[:, 0 : KT * BPC]               # spT own graphs [128, 15*8]
            nf = acts[:, KT * BPC : KT * BPC + NODES]  # nfT own graphs [128, 72]

            # ---- spec MLP layer 1: b-major, W1 moving, accumulate in PSUM ----
            h1bs = cpool.tile([BPC, HIDP], f32)
            if True:
                h1p0 = ph1.tile([BPC, HHALF], f32, tag="h1b")
                h1p1 = ph1.tile([BPC, HHALF], f32, tag="h1b")
                for k in range(KT):
                    kk = 128 if k < KT - 1 else 10  # last tile: 9 spec rows + bias row
                    wsl = w1ts[k][:]
                    lhs = sp[:kk, BPC * k : BPC * (k + 1)]
                    nc.tensor.matmul(h1p0[:], lhs, wsl[:, :HHALF],
                                     start=(k == 0), stop=(k == KT - 1))
                    nc.tensor.matmul(h1p1[:], lhs, wsl[:, HHALF:],
                                     start=(k == 0), stop=(k == KT - 1))
                # relu (bias already folded via the constant-1 spec row)
                nc.vector.tensor_scalar_max(h1bs[:, :HHALF], h1p0[:], 0.0)
                nc.vector.tensor_scalar_max(h1bs[:, HHALF:], h1p1[:], 0.0)

            if True:
                # ---- transpose h1 to hid-major: 8x PE transpose [8,113]->[113,8] ----
                eye = consts[:BPC, OFF_EYE : OFF_EYE + BPC]
                h1ts = cpool.tile([MCH, 64], f32)
                for j in range(8):
                    tp = ptr.tile([MCH, BPC], f32, tag="tr")
                    nc.tensor.transpose(tp[:], h1bs[:, MCH * j : MCH * (j + 1)], eye)
                    nc.vector.tensor_copy(h1ts[:, BPC * j : BPC * (j + 1)], tp[:])

                # ---- layer 2: sT[q, b] accumulated over 8 hid chunks ----
                sps = psh.tile([SC, BPC], f32, tag="sh")
                for j in range(8):
                    nc.tensor.matmul(
                        sps[:],
                        consts[:MCH, OFF_W2T + SC * j : OFF_W2T + SC * (j + 1)],
                        h1ts[:, BPC * j : BPC * (j + 1)],
                        start=(j == 0), stop=(j == 7),
                    )
                sTs = cpool.tile([SC, BPC], f32)
                nc.vector.tensor_scalar(sTs[:], sps[:], consts[:SC, OFF_B2 : OFF_B2 + 1],
                                        0.0, op0=Alu.add, op1=Alu.max)

                # ---- value head ----
                ro = cpool.tile([128, BPC], f32)
                nc.vector.reduce_sum(ro[:], nf.rearrange("p (b n) -> p b n", n=NPG),
                                     axis=mybir.AxisListType.X)
                y1 = psh.tile([64, BPC], f32, tag="sh")
                nc.tensor.matmul(y1[:], consts[:, OFF_WV1 : OFF_WV1 + 64], ro[:], start=True, stop=False)
                nc.tensor.matmul(y1[:], consts[:SC, OFF_WV1 + 64 : OFF_WV1 + 128], sTs[:], start=False, stop=True)
                y1s = cpool.tile([64, BPC], f32)
                nc.vector.tensor_scalar(y1s[:], y1[:], consts[:64, OFF_BV1 : OFF_BV1 + 1],
                                        0.0, op0=Alu.add, op1=Alu.max)
                vps = psh.tile([1, BPC], f32, tag="sh")
                nc.tensor.matmul(vps[:], consts[:64, OFF_WV2 : OFF_WV2 + 1], y1s[:], start=True, stop=True)
                vs = cpool.tile([1, BPC], f32)
                nc.vector.tensor_scalar_add(vs[:], vps[:], consts[:1, OFF_BV2 : OFF_BV2 + 1])
                nc.scalar.dma_start(outv_d[:], vs[:])

                # ---- pair features: hT[c, (b,i,j)] ----
                nfr = cpool.tile([128, NODES], f32)
                nc.vector.tensor_scalar_max(nfr[:], nf, 0.0)
                aips = pab.tile([128, NODES], f32, tag="aips")
                nc.tensor.matmul(aips[:], consts[:, OFF_WA2A : OFF_WA2A + 128], nfr[:], start=True, stop=True)
                bjps = pab.tile([128, NODES], f32, tag="bjps")
                nc.tensor.matmul(bjps[:], consts[:, OFF_WA2B : OFF_WA2B + 128], nfr[:], start=True, stop=True)
                bjs = cpool.tile([128, NODES], f32)
                nc.vector.tensor_copy(bjs[:], bjps[:])
                dps = psh.tile([128, BPC], f32, tag="sh")
                nc.tensor.matmul(dps[:], consts[:SC, OFF_WA2C : OFF_WA2C + 128], sTs[:], start=True, stop=True)
                dt2 = cpool.tile([128, BPC], f32)
                nc.vector.tensor_scalar_add(dt2[:], dps[:], consts[:, OFF_BA2 : OFF_BA2 + 1])
                ai2 = cpool.tile([128, NODES], f32)
                nc.vector.tensor_tensor(
                    ai2[:].rearrange("p (b i) -> p b i", i=NPG),
                    aips[:].rearrange("p (b i) -> p b i", i=NPG),
                    dt2[:].unsqueeze(2).broadcast_to([128, BPC, NPG]),
                    op=Alu.add,
                )
                hT = cpool.tile([128, PAIRS], f32)
                nc.vector.tensor_tensor(
                    hT[:].rearrange("p (b i j) -> p b i j", i=NPG, j=NPG),
                    ai2[:].rearrange("p (b i) -> p b i", i=NPG).unsqueeze(3).broadcast_to([128, BPC, NPG, NPG]),
                    bjs[:].rearrange("p (b j) -> p b j", j=NPG).unsqueeze(2).broadcast_to([128, BPC, NPG, NPG]),
                    op=Alu.add,
                )
                nc.vector.tensor_scalar_max(hT[:], hT[:], 0.0)

                # ---- saf: fp[t, pair] = Wf.T @ relu(hT) + bf ----
                fp1 = ptr.tile([BOND, PAIRS // 2], f32, tag="tr")
                fp2 = ptr.tile([BOND, PAIRS // 2], f32, tag="tr")
                nc.tensor.matmul(fp1[:], consts[:, OFF_WF : OFF_WF + BOND], hT[:, : PAIRS // 2], start=True, stop=True)
                nc.tensor.matmul(fp2[:], consts[:, OFF_WF : OFF_WF + BOND], hT[:, PAIRS // 2 :], start=True, stop=True)
                fps = cpool.tile([BOND, PAIRS], f32)
                nc.vector.tensor_scalar_add(fps[:, : PAIRS // 2], fp1[:], consts[:BOND, OFF_BF : OFF_BF + 1])
                nc.vector.tensor_scalar_add(fps[:, PAIRS // 2 :], fp2[:], consts[:BOND, OFF_BF : OFF_BF + 1])

                # regroup [3, 648] -> X[16b, t*81+e] via a DRAM bounce
                nc.sync.dma_start(fp_d[:], fps[:])
                x_out = bass.AP(Xt[:].tensor, Xt[:].offset, [[16 * ASL, BPC], [1, ASL]])
                x_in = bass.AP(fp_d[:].tensor, 0, [[81, BPC], [PAIRS, BOND], [1, 81]])
                nc.scalar.dma_start(x_out, x_in)

                # ---- gather + masked softmax ----
                G = cpool.tile([128, 256], f32)
                nc.gpsimd.ap_gather(G[:], Xt[:], idxs[:], channels=128, num_elems=ASL, d=1, num_idxs=256)
                X2 = cpool.tile([128, ASL], f32)
                nc.vector.tensor_tensor(X2[:], G[:, :ASL], Mt[:], op=Alu.add)
                nmx = cpool.tile([128, 1], f32)
                nc.vector.reduce_max(nmx[:], X2[:], axis=mybir.AxisListType.X, negate=True)
                E = cpool.tile([128, ASL], f32)
                sums = cpool.tile([128, 1], f32)
                nc.scalar.activation(E[:], X2[:], Act.Exp, bias=nmx[:], accum_out=sums[:])
                rc = cpool.tile([128, 1], f32)
                nc.vector.reciprocal(rc[:], sums[:])
                OU = cpool.tile([128, ASL], f32)
                nc.vector.tensor_scalar_mul(OU[:], E[:], rc[:])
                o_in = bass.AP(OU[:].tensor, OU[:].offset, [[16 * ASL, BPC], [1, ASL]])
                nc.sync.dma_start(outp_d[:], o_in)

                if DEBUG_TAPS:
                    taps = {
                        "t_h1bs": h1bs, "t_h1ts": h1ts, "t_sTs": sTs, "t_ro": ro,
                        "t_y1s": y1s, "t_nfr": nfr, "t_dt2": dt2, "t_ai2": ai2,
                        "t_bjs": bjs, "t_hT": hT, "t_fps": fps, "t_Xt": Xt,
                        "t_G": G, "t_X2": X2, "t_sums": sums,
                    }
                    for tname, ttile in taps.items():
                        shp = list(ttile[:].shape)
                        td = nc.declare_dram_parameter(tname, shp, f32, isOutput=True)
                        nc.sync.dma_start(td[:], ttile[:])

    nc.compile()
    return nc


def _marshal(node_features, specs, mask, indexmask, W1, b1, W2, b2,
             Wv1, bv1, Wv2, bv2, Wa2, ba2, Wf, bf):
    """Host-side sharding + layout packing. Returns in_maps (one per core)."""
    # W1 with b1 folded as an extra (constant-1-input) row, k-tile packed
    w1p = np.zeros((1802, HIDP), np.float32)
    w1p[:SL, :HID] = W1
    w1p[SL, :HID] = b1
    w1ks = {f"w1k{k}": np.ascontiguousarray(w1p[128 * k : 128 * k + (128 if k < KT - 1 else 10)])
            for k in range(KT)}
    w2p = np.zeros((HIDP, SC), np.float32)
    w2p[:HID] = W2

    consts = np.zeros((128, CF), np.float32)
    consts[:, OFF_WA2A : OFF_WA2A + 128] = Wa2[0:128]
    consts[:, OFF_WA2B : OFF_WA2B + 128] = Wa2[128:256]
    consts[:, OFF_WV1 : OFF_WV1 + 64] = Wv1[0:128]
    consts[:100, OFF_WV1 + 64 : OFF_WV1 + 128] = Wv1[128:228]
    consts[:100, OFF_WA2C : OFF_WA2C + 128] = Wa2[256:356]
    consts[:64, OFF_WV2] = Wv2[:, 0]
    consts[:100, OFF_B2] = b2
    consts[:64, OFF_BV1] = bv1
    consts[:1, OFF_BV2] = bv2
    consts[:, OFF_BA2] = ba2
    consts[:BOND, OFF_BF] = bf
    consts[:, OFF_WF : OFF_WF + BOND] = Wf
    consts[:BPC, OFF_EYE : OFF_EYE + BPC] = np.eye(BPC, dtype=np.float32)
    consts[:MCH, OFF_W2T : OFF_W2T + 800] = w2p.reshape(8, MCH, SC).transpose(1, 0, 2).reshape(MCH, 800)

    # index remap to device fp layout (t-major): v -> (v%3)*81 + v//3,
    # then wrap per-graph lists across the 16 partitions of its group
    v = indexmask.astype(np.int64)
    newidx = ((v % BOND) * (NPG * NPG) + v // BOND).astype(np.int16)  # [64, 243]

    in_maps = []
    for c in range(NCORES):
        gsl = slice(c * BPC, (c + 1) * BPC)
        nsl = slice(c * NODES, (c + 1) * NODES)
        # spec transposed + k-tiled + constant-1 bias row (row 1801)
        spc = np.zeros((BPC, KT * 128), np.float32)
        spc[:, :SL] = specs[gsl, 0, :]
        spc[:, SL] = 1.0
        spT = spc.reshape(BPC, KT, 128).transpose(2, 1, 0).reshape(128, KT * BPC)
        acts = np.zeros((128, KT * BPC + NODES), np.float32)
        acts[:, 0 : KT * BPC] = spT
        acts[:, KT * BPC :] = node_features[nsl].T
        padidx = np.zeros((BPC, 256), np.int16)
        padidx[:, :ASL] = newidx[gsl]
        idx16 = padidx.reshape(BPC, 16, 16).transpose(0, 2, 1).reshape(128, 16)
        in_maps.append({
            "consts": consts,
            "acts": acts,
            **w1ks,
            "mask8": np.ascontiguousarray(mask[gsl], np.float32),
            "idx16": np.ascontiguousarray(idx16),
        })
    return in_maps


def _run(inputs, trace=False):
    from concourse.bass_utils import run_bass_kernel_spmd

    if "nc" not in _CACHE:
        _CACHE["nc"] = _build_nc()
    nc = _CACHE["nc"]

    in_maps = _marshal(
        _f32(inputs["node_features"]), _f32(inputs["specs"]),
        _f32(inputs["mask"]), np.asarray(inputs["indexmask"]),
        _f32(inputs["W1"]), _f32(inputs["b1"]), _f32(inputs["W2"]), _f32(inputs["b2"]),
        _f32(inputs["Wv1"]), _f32(inputs["bv1"]), _f32(inputs["Wv2"]), _f32(inputs["bv2"]),
        _f32(inputs["Wa2"]), _f32(inputs["ba2"]), _f32(inputs["Wf"]), _f32(inputs["bf"]),
    )
    res = run_bass_kernel_spmd(nc, in_maps, core_ids=list(range(NCORES)), trace=trace)
    probs = np.concatenate([res.results[c]["out_p"] for c in range(NCORES)], axis=0)
    v = np.concatenate([res.results[c]["out_v"][0] for c in range(NCORES)])[:, None]
    return (probs, v.astype(np.float32)), res


def kernel(**inputs):
    (probs, v), _ = _run(inputs, trace=False)
    return probs, v


# revision 19
# speedup vs baseline: 2.0291x; 1.0276x over previous
"""Trainium2 Bass kernel for nn_ActionPredictionModel (scatter_memory).

Data-parallel over graphs: 8 graphs (72 nodes) per NeuronCore, weights
replicated (collectives measured ~80us launch-skew here, so none used).
Per core:
  - spec MLP layer 1 with W1 as the *moving* operand (b-major output):
    fp32 PE cost is ~2.2ns/col of moving stream, vs ~3.7ns/col as
    stationary, so stationary is the tiny spec tile (8 cols) and all of
    W1 streams through as rhs. b1 is folded in via a constant-1 spec row.
    h1 [8, 904] is then relu'd and PE-transposed (8x [8,113]->[113,8])
    for the hid-major layer 2.
  - value head (sum-pool readout + spec -> scalar)
  - pair action features; block-diagonal structure: only the 9x9
    same-graph pair blocks are materialized ([128ch, 648pairs])
  - per-graph flatten + indexmask gather (gpsimd ap_gather, ucode
    warmed early) + softmax (DVE + one ACT Exp, table warmed early)
Host does only sharding/layout marshalling (transpose, pad, tile-pack,
index remap to the on-device fp layout) and output concatenation.
"""

import numpy as np

# problem dims (hardcoded per contract)
B, NPG, H = 64, 9, 128
SL, SC, BOND, ASL = 1801, 100, 3, 243
NCORES = 8
BPC = B // NCORES            # graphs per core = 8
NODES = BPC * NPG            # nodes per core = 72
PAIRS = BPC * NPG * NPG      # same-graph pairs per core = 648

KT = 15                      # k-tiles over spec dim (14*128 + 10(incl bias row))
HID = 900
HIDP = 904                   # padded hidden (8 * 113)
MCH = HIDP // 8              # hid chunk for transposes / L2 = 113
HHALF = HIDP // 2            # 452 (psum bank-sized moving chunks)

# consts column offsets
OFF_WA2A, OFF_WA2B, OFF_WV1, OFF_WA2C = 0, 128, 256, 384
OFF_WV2, OFF_B2, OFF_BV1, OFF_BV2, OFF_BA2, OFF_BF = 512, 513, 514, 515, 516, 517
OFF_WF, OFF_EYE, OFF_W2T = 518, 521, 529
CF = 529 + 800

_CACHE = {}
DEBUG_TAPS = False


def _f32(x):
    return np.ascontiguousarray(np.asarray(x), dtype=np.float32)


def _build_nc():
    import concourse.mybir as mybir
    import concourse.tile as tile
    import concourse.bacc as bacc
    import concourse.bass as bass

    f32 = mybir.dt.float32
    i16 = mybir.dt.int16
    Alu = mybir.AluOpType
    Act = mybir.ActivationFunctionType

    nc = bacc.Bacc("TRN2", target_bir_lowering=False, debug=False, num_devices=1)

    consts_d = nc.declare_dram_parameter("consts", [128, CF], f32, isOutput=False)
    acts_d = nc.declare_dram_parameter("acts", [128, KT * BPC + NODES], f32, isOutput=False)
    w1k_d = [nc.declare_dram_parameter(f"w1k{k}", [128 if k < KT - 1 else 10, HIDP], f32, isOutput=False)
             for k in range(KT)]
    mask_d = nc.declare_dram_parameter("mask8", [BPC, ASL], f32, isOutput=False)
    idx_d = nc.declare_dram_parameter("idx16", [128, 16], i16, isOutput=False)
    outp_d = nc.declare_dram_parameter("out_p", [BPC, ASL], f32, isOutput=True)
    outv_d = nc.declare_dram_parameter("out_v", [1, BPC], f32, isOutput=True)

    with tile.TileContext(nc) as tc:
        with (
            tc.tile_pool(name="cpool", bufs=1) as cpool,
            tc.tile_pool(name="pab", bufs=1, space="PSUM") as pab,
            tc.tile_pool(name="psh", bufs=2, space="PSUM") as psh,
            tc.tile_pool(name="ph1", bufs=2, space="PSUM") as ph1,
            tc.tile_pool(name="ptr", bufs=2, space="PSUM") as ptr,
        ):
            # ---- input loads: acts + per-k W1 chunks gate the stream ----
            acts = cpool.tile([128, KT * BPC + NODES], f32)
            nc.scalar.dma_start(acts[:], acts_d[:])
            idxs = cpool.tile([128, 16], i16)
            nc.sync.dma_start(idxs[:], idx_d[:])
            w1ts = []
            for k in range(KT):
                kk = 128 if k < KT - 1 else 10
                wt = cpool.tile([kk, HIDP], f32, tag=f"w1k{k}")
                eng = nc.sync if k % 2 == 0 else nc.scalar
                eng.dma_start(wt[:], w1k_d[k][:])
                w1ts.append(wt)
            consts = cpool.tile([128, CF], f32)
            nc.sync.dma_start(consts[:], consts_d[:])

            # mask / gather-source tiles (memset first: only rows 16*b are real)
            Xt = cpool.tile([128, ASL], f32, tag="Xt")
            Mt = cpool.tile([128, ASL], f32, tag="Mt")
            nc.vector.memset(Xt[:], 0.0)
            nc.vector.memset(Mt[:], 0.0)
            m_out = bass.AP(Mt[:].tensor, Mt[:].offset, [[16 * ASL, BPC], [1, ASL]])
            nc.scalar.dma_start(m_out, mask_d[:])

            # warm-ups: ACT Exp table + gpsimd ap_gather ucode (hide under stream)
            warm = cpool.tile([1, 1], f32)
            nc.vector.memset(warm[:], 0.0)
            warmo = cpool.tile([1, 1], f32)
            nc.scalar.activation(warmo[:], warm[:], Act.Exp)
            gwi = cpool.tile([16, 4], f32)
            gwx = cpool.tile([16, 1], i16)
            gwo = cpool.tile([16, 16], f32)
            nc.vector.memset(gwi[:], 0.0)
            nc.vector.memset(gwx[:], 0)
            nc.gpsimd.ap_gather(gwo[:], gwi[:], gwx[:], channels=16, num_elems=4, d=1, num_idxs=16)

            sp = acts[:, 0 : KT * BPC]               # spT own graphs [128, 15*8]
            nf = acts[:, KT * BPC : KT * BPC + NODES]  # nfT own graphs [128, 72]

            # ---- spec MLP layer 1: b-major, W1 moving, accumulate in PSUM ----
            h1bs = cpool.tile([BPC, HIDP], f32)
            if True:
                h1p0 = ph1.tile([BPC, HHALF], f32, tag="h1b")
                h1p1 = ph1.tile([BPC, HHALF], f32, tag="h1b")
                for k in range(KT):
                    kk = 128 if k < KT - 1 else 10  # last tile: 9 spec rows + bias row
                    wsl = w1ts[k][:]
                    lhs = sp[:kk, BPC * k : BPC * (k + 1)]
                    nc.tensor.matmul(h1p0[:], lhs, wsl[:, :HHALF],
                                     start=(k == 0), stop=(k == KT - 1))
                    nc.tensor.matmul(h1p1[:], lhs, wsl[:, HHALF:],
                                     start=(k == 0), stop=(k == KT - 1))
                # relu (bias already folded via the constant-1 spec row)
                nc.vector.tensor_scalar_max(h1bs[:, :HHALF], h1p0[:], 0.0)
                nc.vector.tensor_scalar_max(h1bs[:, HHALF:], h1p1[:], 0.0)

            if True:
                # ---- transpose h1 to hid-major: 8x PE transpose [8,113]->[113,8] ----
                eye = consts[:BPC, OFF_EYE : OFF_EYE + BPC]
                h1ts = cpool.tile([MCH, 64], f32)
                for j in range(8):
                    tp = ptr.tile([MCH, BPC], f32, tag="tr")
                    nc.tensor.transpose(tp[:], h1bs[:, MCH * j : MCH * (j + 1)], eye)
                    nc.vector.tensor_copy(h1ts[:, BPC * j : BPC * (j + 1)], tp[:])

                # ---- layer 2: sT[q, b] accumulated over 8 hid chunks ----
                sps = psh.tile([SC, BPC], f32, tag="sh")
                for j in range(8):
                    nc.tensor.matmul(
                        sps[:],
                        consts[:MCH, OFF_W2T + SC * j : OFF_W2T + SC * (j + 1)],
                        h1ts[:, BPC * j : BPC * (j + 1)],
                        start=(j == 0), stop=(j == 7),
                    )
                sTs = cpool.tile([SC + 1, BPC], f32)
                nc.vector.memset(sTs[:], 1.0)
                nc.vector.tensor_scalar(sTs[:SC, :], sps[:], consts[:SC, OFF_B2 : OFF_B2 + 1],
                                        0.0, op0=Alu.add, op1=Alu.max)

                # ---- value head ----
                ro = cpool.tile([128, BPC], f32)
                nc.vector.reduce_sum(ro[:], nf.rearrange("p (b n) -> p b n", n=NPG),
                                     axis=mybir.AxisListType.X)
                y1 = psh.tile([64, BPC], f32, tag="sh")
                nc.tensor.matmul(y1[:], consts[:, OFF_WV1 : OFF_WV1 + 64], ro[:], start=True, stop=False)
                nc.tensor.matmul(y1[:], consts[:SC, OFF_WV1 + 64 : OFF_WV1 + 128], sTs[:SC, :], start=False, stop=True)
                y1s = cpool.tile([64, BPC], f32)
                nc.vector.tensor_scalar(y1s[:], y1[:], consts[:64, OFF_BV1 : OFF_BV1 + 1],
                                        0.0, op0=Alu.add, op1=Alu.max)
                vps = psh.tile([1, BPC], f32, tag="sh")
                nc.tensor.matmul(vps[:], consts[:64, OFF_WV2 : OFF_WV2 + 1], y1s[:], start=True, stop=True)
                vs = cpool.tile([1, BPC], f32)
                nc.vector.tensor_scalar_add(vs[:], vps[:], consts[:1, OFF_BV2 : OFF_BV2 + 1])
                nc.scalar.dma_start(outv_d[:], vs[:])

                # ---- pair features: hT[c, (b,i,j)] ----
                nfr = cpool.tile([128, NODES], f32)
                nc.vector.tensor_scalar_max(nfr[:], nf, 0.0)
                aips = pab.tile([128, NODES], f32, tag="aips")
                nc.tensor.matmul(aips[:], consts[:, OFF_WA2A : OFF_WA2A + 128], nfr[:], start=True, stop=True)
                bjps = pab.tile([128, NODES], f32, tag="bjps")
                nc.tensor.matmul(bjps[:], consts[:, OFF_WA2B : OFF_WA2B + 128], nfr[:], start=True, stop=True)
                bjs = cpool.tile([128, NODES], f32)
                nc.vector.tensor_copy(bjs[:], bjps[:])
                dps = psh.tile([128, BPC], f32, tag="sh")
                nc.tensor.matmul(dps[:], consts[: SC + 1, OFF_WA2C : OFF_WA2C + 128], sTs[:], start=True, stop=True)
                dt2 = cpool.tile([128, BPC], f32)
                nc.vector.tensor_copy(dt2[:], dps[:])
                ai2 = cpool.tile([128, NODES], f32)
                nc.vector.tensor_tensor(
                    ai2[:].rearrange("p (b i) -> p b i", i=NPG),
                    aips[:].rearrange("p (b i) -> p b i", i=NPG),
                    dt2[:].unsqueeze(2).broadcast_to([128, BPC, NPG]),
                    op=Alu.add,
                )
                hT = cpool.tile([128, PAIRS], f32)
                nc.vector.tensor_tensor(
                    hT[:].rearrange("p (b i j) -> p b i j", i=NPG, j=NPG),
                    ai2[:].rearrange("p (b i) -> p b i", i=NPG).unsqueeze(3).broadcast_to([128, BPC, NPG, NPG]),
                    bjs[:].rearrange("p (b j) -> p b j", j=NPG).unsqueeze(2).broadcast_to([128, BPC, NPG, NPG]),
                    op=Alu.add,
                )
                nc.vector.tensor_scalar_max(hT[:], hT[:], 0.0)

                # ---- saf: fp[t, pair] = Wf.T @ relu(hT) + bf ----
                fp1 = ptr.tile([BOND, PAIRS // 2], f32, tag="tr")
                fp2 = ptr.tile([BOND, PAIRS // 2], f32, tag="tr")
                nc.tensor.matmul(fp1[:], consts[:, OFF_WF : OFF_WF + BOND], hT[:, : PAIRS // 2], start=True, stop=True)
                nc.tensor.matmul(fp2[:], consts[:, OFF_WF : OFF_WF + BOND], hT[:, PAIRS // 2 :], start=True, stop=True)
                fps = cpool.tile([BOND, PAIRS], f32)
                nc.vector.tensor_scalar_add(fps[:, : PAIRS // 2], fp1[:], consts[:BOND, OFF_BF : OFF_BF + 1])
                nc.vector.tensor_scalar_add(fps[:, PAIRS // 2 :], fp2[:], consts[:BOND, OFF_BF : OFF_BF + 1])

                # regroup [3, 648] -> X[16b, t*81+e]: one SBUF->SBUF DMA per
                # bond plane (src = single partition, contiguous; dst strided)
                NN = NPG * NPG
                for t in range(BOND):
                    x_out = bass.AP(Xt[:].tensor, Xt[:].offset + NN * t,
                                    [[16 * ASL, BPC], [1, NN]])
                    eng = (nc.sync, nc.scalar, nc.sync)[t]
                    eng.dma_start(x_out, fps[t : t + 1, :].rearrange("p (b e) -> p b e", e=NN))

                # ---- gather + masked softmax ----
                G = cpool.tile([128, 256], f32)
                nc.gpsimd.ap_gather(G[:], Xt[:], idxs[:], channels=128, num_elems=ASL, d=1, num_idxs=256)
                X2 = cpool.tile([128, ASL], f32)
                nc.vector.tensor_tensor(X2[:], G[:, :ASL], Mt[:], op=Alu.add)
                nmx = cpool.tile([128, 1], f32)
                nc.vector.reduce_max(nmx[:], X2[:], axis=mybir.AxisListType.X, negate=True)
                E = cpool.tile([128, ASL], f32)
                sums = cpool.tile([128, 1], f32)
                nc.scalar.activation(E[:], X2[:], Act.Exp, bias=nmx[:], accum_out=sums[:])
                rc = cpool.tile([128, 1], f32)
                nc.vector.reciprocal(rc[:], sums[:])
                OU = cpool.tile([128, ASL], f32)
                nc.vector.tensor_scalar_mul(OU[:], E[:], rc[:])
                o_in = bass.AP(OU[:].tensor, OU[:].offset, [[16 * ASL, BPC], [1, ASL]])
                nc.sync.dma_start(outp_d[:], o_in)

                if DEBUG_TAPS:
                    taps = {
                        "t_h1bs": h1bs, "t_h1ts": h1ts, "t_sTs": sTs, "t_ro": ro,
                        "t_y1s": y1s, "t_nfr": nfr, "t_ai2": ai2,
                        "t_bjs": bjs, "t_hT": hT, "t_fps": fps, "t_Xt": Xt,
                        "t_G": G, "t_X2": X2, "t_sums": sums,
                    }
                    for tname, ttile in taps.items():
                        shp = list(ttile[:].shape)
                        td = nc.declare_dram_parameter(tname, shp, f32, isOutput=True)
                        nc.sync.dma_start(td[:], ttile[:])

    nc.compile()
    return nc


def _marshal(node_features, specs, mask, indexmask, W1, b1, W2, b2,
             Wv1, bv1, Wv2, bv2, Wa2, ba2, Wf, bf):
    """Host-side sharding + layout packing. Returns in_maps (one per core)."""
    # W1 with b1 folded as an extra (constant-1-input) row, k-tile packed
    w1p = np.zeros((1802, HIDP), np.float32)
    w1p[:SL, :HID] = W1
    w1p[SL, :HID] = b1
    w1ks = {f"w1k{k}": np.ascontiguousarray(w1p[128 * k : 128 * k + (128 if k < KT - 1 else 10)])
            for k in range(KT)}
    w2p = np.zeros((HIDP, SC), np.float32)
    w2p[:HID] = W2

    consts = np.zeros((128, CF), np.float32)
    consts[:, OFF_WA2A : OFF_WA2A + 128] = Wa2[0:128]
    consts[:, OFF_WA2B : OFF_WA2B + 128] = Wa2[128:256]
    consts[:, OFF_WV1 : OFF_WV1 + 64] = Wv1[0:128]
    consts[:100, OFF_WV1 + 64 : OFF_WV1 + 128] = Wv1[128:228]
    consts[:100, OFF_WA2C : OFF_WA2C + 128] = Wa2[256:356]
    consts[100, OFF_WA2C : OFF_WA2C + 128] = ba2
    consts[:64, OFF_WV2] = Wv2[:, 0]
    consts[:100, OFF_B2] = b2
    consts[:64, OFF_BV1] = bv1
    consts[:1, OFF_BV2] = bv2
    consts[:, OFF_BA2] = ba2
    consts[:BOND, OFF_BF] = bf
    consts[:, OFF_WF : OFF_WF + BOND] = Wf
    consts[:BPC, OFF_EYE : OFF_EYE + BPC] = np.eye(BPC, dtype=np.float32)
    consts[:MCH, OFF_W2T : OFF_W2T + 800] = w2p.reshape(8, MCH, SC).transpose(1, 0, 2).reshape(MCH, 800)

    # index remap to device fp layout (t-major): v -> (v%3)*81 + v//3,
    # then wrap per-graph lists across the 16 partitions of its group
    v = indexmask.astype(np.int64)
    newidx = ((v % BOND) * (NPG * NPG) + v // BOND).astype(np.int16)  # [64, 243]

    in_maps = []
    for c in range(NCORES):
        gsl = slice(c * BPC, (c + 1) * BPC)
        nsl = slice(c * NODES, (c + 1) * NODES)
        # spec transposed + k-tiled + constant-1 bias row (row 1801)
        spc = np.zeros((BPC, KT * 128), np.float32)
        spc[:, :SL] = specs[gsl, 0, :]
        spc[:, SL] = 1.0
        spT = spc.reshape(BPC, KT, 128).transpose(2, 1, 0).reshape(128, KT * BPC)
        acts = np.zeros((128, KT * BPC + NODES), np.float32)
        acts[:, 0 : KT * BPC] = spT
        acts[:, KT * BPC :] = node_features[nsl].T
        padidx = np.zeros((BPC, 256), np.int16)
        padidx[:, :ASL] = newidx[gsl]
        idx16 = padidx.reshape(BPC, 16, 16).transpose(0, 2, 1).reshape(128, 16)
        in_maps.append({
            "consts": consts,
            "acts": acts,
            **w1ks,
            "mask8": np.ascontiguousarray(mask[gsl], np.float32),
            "idx16": np.ascontiguousarray(idx16),
        })
    return in_maps


def _run(inputs, trace=False):
    from concourse.bass_utils import run_bass_kernel_spmd

    if "nc" not in _CACHE:
        _CACHE["nc"] = _build_nc()
    nc = _CACHE["nc"]

    in_maps = _marshal(
        _f32(inputs["node_features"]), _f32(inputs["specs"]),
        _f32(inputs["mask"]), np.asarray(inputs["indexmask"]),
        _f32(inputs["W1"]), _f32(inputs["b1"]), _f32(inputs["W2"]), _f32(inputs["b2"]),
        _f32(inputs["Wv1"]), _f32(inputs["bv1"]), _f32(inputs["Wv2"]), _f32(inputs["bv2"]),
        _f32(inputs["Wa2"]), _f32(inputs["ba2"]), _f32(inputs["Wf"]), _f32(inputs["bf"]),
    )
    res = run_bass_kernel_spmd(nc, in_maps, core_ids=list(range(NCORES)), trace=trace)
    probs = np.concatenate([res.results[c]["out_p"] for c in range(NCORES)], axis=0)
    v = np.concatenate([res.results[c]["out_v"][0] for c in range(NCORES)])[:, None]
    return (probs, v.astype(np.float32)), res


def kernel(**inputs):
    (probs, v), _ = _run(inputs, trace=False)
    return probs, v
